# revision 6
# baseline (speedup 1.0000x reference)
"""PhaseSynchronizedAttention on 8 TRN2 NeuronCores.

Math: the per-head phase rotation is an orthogonal transform applied to both
q-heads and k-heads; it cancels exactly in q·k and v is untouched, so the
module is standard multi-head attention.  The 1/sqrt(d_k) score scale is
folded into the Q projection weights on the host.

Sharding: tensor-parallel over heads — 16 heads / 8 cores = 2 heads per core.
Each core computes its head-slice of the Q/K/V projections, the attention for
its two heads, and a partial output projection  y_c = out_c @ Wo[:, c].T ;
the host sums the 8 partials and adds bo (the unshard for a sum-sharded
output).

Device layouts (all activations transposed so no on-device activation
transposes are needed except one [128,4096] PE transpose of vh):
  qhT/khT/vhT [128 head-dims, 4096 rows] = W_shard.T.T @ xT
  scores^T    [mk, mq] tiles = khT-slice.T @ qhT-slice  (two heads packed on
              disjoint PE row-groups, K=64 each)
  P = exp(scores^T) via ScalarE during PSUM evacuation (softmax max-shift is
              unnecessary: scores are N(0,1)-ish, |s| < ~10 << 88)
  AV: out^T = vh_aug.T @ P with vh_aug = [vh | ones] so the attention row
              sums ride along as PSUM row 64; normalization is deferred to
              out^T (row-scale commutes through the output projection)
  y = outT.T @ WoT-slice
"""

import math
import sys

if "/opt/trn_rl_repo" not in sys.path:
    sys.path.insert(0, "/opt/trn_rl_repo")

import numpy as np

B, S, D_MODEL, N_HEADS, D_K = 2, 2048, 1024, 16, 64
M = B * S                      # 4096 flattened rows
N_CORES = 8
HPC = N_HEADS // N_CORES       # heads per core = 2
NH = HPC * D_K                 # head-dims per core = 128
KT = D_MODEL // 128            # 8 contraction tiles
NMB = M // 512                 # 8 row-blocks of 512
NKV = S // 128                 # 16 kv tiles per batch
F32 = None                     # set on first build (mybir import)

_cache = {}


def _build_nc():
    from contextlib import ExitStack

    import concourse.bass as bass
    from concourse import bacc, masks, mybir, tile

    f32 = mybir.dt.float32
    nc = bacc.Bacc()

    qT_d = nc.dram_tensor("qT", [D_MODEL, M], f32, kind="ExternalInput")
    kT_d = nc.dram_tensor("kT", [D_MODEL, M], f32, kind="ExternalInput")
    vT_d = nc.dram_tensor("vT", [D_MODEL, M], f32, kind="ExternalInput")
    wqT_d = nc.dram_tensor("wqT", [D_MODEL, NH], f32, kind="ExternalInput")
    wkT_d = nc.dram_tensor("wkT", [D_MODEL, NH], f32, kind="ExternalInput")
    wvT_d = nc.dram_tensor("wvT", [D_MODEL, NH], f32, kind="ExternalInput")
    bq_d = nc.dram_tensor("bq", [NH, 1], f32, kind="ExternalInput")
    bk_d = nc.dram_tensor("bk", [NH, 1], f32, kind="ExternalInput")
    bv_d = nc.dram_tensor("bv", [NH, 1], f32, kind="ExternalInput")
    woT_d = nc.dram_tensor("woT", [NH, D_MODEL], f32, kind="ExternalInput")
    y_d = nc.dram_tensor("y", [M, D_MODEL], f32, kind="ExternalOutput")

    with tile.TileContext(nc) as tc, ExitStack() as ctx:
        singles = ctx.enter_context(tc.tile_pool(name="singles", bufs=1))

        # persistent SBUF state
        w_sb = {}
        b_sb = {}
        for name, wd, bd in (
            ("q", wqT_d, bq_d),
            ("k", wkT_d, bk_d),
            ("v", wvT_d, bv_d),
        ):
            w = singles.tile([128, KT, NH], f32, tag=f"w_{name}")
            nc.sync.dma_start(out=w[:], in_=wd.rearrange("(t p) n -> p t n", p=128))
            b = singles.tile([NH, 1], f32, tag=f"b_{name}")
            nc.sync.dma_start(out=b[:], in_=bd[:])
            w_sb[name], b_sb[name] = w, b
        wo_sb = singles.tile([NH, D_MODEL], f32, tag="wo")
        nc.sync.dma_start(out=wo_sb[:], in_=woT_d[:])
        identity = singles.tile([128, 128], f32, tag="ident")
        masks.make_identity(nc, identity[:])

        qhT = singles.tile([128, M], f32, tag="qhT")
        khT = singles.tile([128, M], f32, tag="khT")
        vhT = singles.tile([128, M], f32, tag="vhT")
        outT = singles.tile([128, M], f32, tag="outT")
        vh_aug = singles.tile([128, M // 128, 130], f32, tag="vh_aug")
        nc.vector.memset(vh_aug[:], 1.0)
        bcast = singles.tile([128, M], f32, tag="bcast")

        # ---- Phase A: projections qhT/khT/vhT = W.T.T @ xT ----
        with (
            tc.tile_pool(name="xin", bufs=3) as xin_pool,
            tc.tile_pool(name="proj_ps", bufs=8, space="PSUM") as pp,
        ):
            for name, src, dst in (
                ("q", qT_d, qhT),
                ("k", kT_d, khT),
                ("v", vT_d, vhT),
            ):
                ps = [pp.tile([128, 512], f32, tag="pps", name=f"pps{_i}") for _i in range(NMB)]
                for kt in range(KT):
                    xt = xin_pool.tile([128, M], f32, tag="xin")
                    nc.sync.dma_start(
                        out=xt[:], in_=src[kt * 128 : (kt + 1) * 128, :]
                    )
                    for mb in range(NMB):
                        nc.tensor.matmul(
                            ps[mb][:],
                            lhsT=w_sb[name][:, kt, :],
                            rhs=xt[:, mb * 512 : (mb + 1) * 512],
                            start=(kt == 0),
                            stop=(kt == KT - 1),
                        )
                for mb in range(NMB):
                    nc.vector.tensor_scalar_add(
                        out=dst[:, mb * 512 : (mb + 1) * 512],
                        in0=ps[mb][:],
                        scalar1=b_sb[name][:],
                    )

        # ---- Phase B: vh_aug[mk-tile] = [vh_h0 | 1 | vh_h1 | 1] ----
        with tc.tile_pool(name="tr_ps", bufs=2, space="PSUM") as tp:
            for t in range(M // 128):
                pst = tp.tile([128, 128], f32, tag="trp")
                nc.tensor.transpose(
                    pst[:], vhT[:, t * 128 : (t + 1) * 128], identity[:]
                )
                nc.vector.tensor_copy(out=vh_aug[:, t, 0:64], in_=pst[:, 0:64])
                nc.vector.tensor_copy(out=vh_aug[:, t, 65:129], in_=pst[:, 64:128])

        # ---- Phase C: attention (scores^T -> exp -> AV+rowsums) ----
        dpool = ctx.enter_context(tc.tile_pool(name="dram", bufs=1, space="DRAM"))
        sums_d = dpool.tile([HPC, M], f32)
        with (
            tc.tile_pool(name="s_ps", bufs=3, space="PSUM") as sp,
            tc.tile_pool(name="av_ps", bufs=2, space="PSUM") as avp,
            tc.tile_pool(name="p_sb", bufs=6) as ppool,
            tc.tile_pool(name="stg_sb", bufs=4) as stgp,
        ):
            for b in range(B):
                for mqb in range(4):
                    qc0 = b * S + mqb * 512
                    avps = [avp.tile([65, 512], f32, tag="av", name=f"av{_i}") for _i in range(HPC)]
                    for g in range(NKV // 2):
                        for i in range(HPC):
                            sps = sp.tile([128, 1024], f32, tag="s")
                            for j in range(2):
                                kr0 = b * S + (2 * g + j) * 128
                                nc.tensor.matmul(
                                    sps[:, j * 512 : (j + 1) * 512],
                                    lhsT=khT[i * 64 : (i + 1) * 64, kr0 : kr0 + 128],
                                    rhs=qhT[i * 64 : (i + 1) * 64, qc0 : qc0 + 512],
                                    start=True,
                                    stop=True,
                                    tile_position=(i * 64, 0),
                                )
                            pt = ppool.tile([128, 1024], f32, tag="p")
                            nc.scalar.activation(
                                out=pt[:],
                                in_=sps[:],
                                func=mybir.ActivationFunctionType.Exp,
                            )
                            for j in range(2):
                                tg = b * NKV + 2 * g + j
                                nc.tensor.matmul(
                                    avps[i][:],
                                    lhsT=vh_aug[:, tg, i * 65 : i * 65 + 65],
                                    rhs=pt[:, j * 512 : (j + 1) * 512],
                                    start=(g == 0 and j == 0),
                                    stop=(g == NKV // 2 - 1 and j == 1),
                                )
                    # DVE lanes can't cross partitions: stage PSUM->SBUF at the
                    # same partitions, then DMA remaps head 1 to rows 64:128.
                    for i in range(HPC):
                        stage = stgp.tile([65, 512], f32, tag="stg", name=f"stg{i}")
                        nc.vector.tensor_copy(out=stage[:], in_=avps[i][:])
                        if i == 0:
                            nc.vector.tensor_copy(
                                out=outT[0:64, qc0 : qc0 + 512], in_=stage[0:64, :]
                            )
                        else:
                            nc.sync.dma_start(
                                out=outT[64:128, qc0 : qc0 + 512], in_=stage[0:64, :]
                            )
                        nc.sync.dma_start(
                            out=sums_d[i : i + 1, qc0 : qc0 + 512],
                            in_=stage[64:65, :],
                        )

        # ---- Phase D: normalize outT, output projection ----
        for i in range(HPC):
            row = sums_d[i : i + 1, :]
            bc_src = bass.AP(
                tensor=row.tensor, offset=row.offset, ap=[[0, 64]] + row.ap[1:]
            )
            nc.sync.dma_start(out=bcast[i * 64 : (i + 1) * 64, :], in_=bc_src)
        nc.vector.reciprocal(out=bcast[:], in_=bcast[:])
        nc.vector.tensor_mul(outT[:], outT[:], bcast[:])

        with (
            tc.tile_pool(name="y_ps", bufs=4, space="PSUM") as yp,
            tc.tile_pool(name="y_sb", bufs=3) as ysb,
        ):
            for mb in range(M // 128):
                yt = ysb.tile([128, D_MODEL], f32, tag="y")
                for jb in range(2):
                    yps = yp.tile([128, 512], f32, tag="yps")
                    nc.tensor.matmul(
                        yps[:],
                        lhsT=outT[:, mb * 128 : (mb + 1) * 128],
                        rhs=wo_sb[:, jb * 512 : (jb + 1) * 512],
                        start=True,
                        stop=True,
                    )
                    nc.vector.tensor_copy(
                        out=yt[:, jb * 512 : (jb + 1) * 512], in_=yps[:]
                    )
                nc.sync.dma_start(
                    out=y_d[mb * 128 : (mb + 1) * 128, :], in_=yt[:]
                )

    nc.finalize()
    return nc


def kernel(q, k, v, Wq, bq, Wk, bk, Wv, bv, Wo, bo, phase=None, **_unused):
    from concourse.bass_utils import run_bass_kernel_spmd

    q = np.asarray(q, np.float32)
    k = np.asarray(k, np.float32)
    v = np.asarray(v, np.float32)
    Wq = np.asarray(Wq, np.float32)
    Wk = np.asarray(Wk, np.float32)
    Wv = np.asarray(Wv, np.float32)
    Wo = np.asarray(Wo, np.float32)
    bq = np.asarray(bq, np.float32)
    bk = np.asarray(bk, np.float32)
    bv = np.asarray(bv, np.float32)
    bo = np.asarray(bo, np.float32)

    if "nc" not in _cache:
        _cache["nc"] = _build_nc()
    nc = _cache["nc"]

    scale = 1.0 / math.sqrt(D_K)
    qT = np.ascontiguousarray(q.reshape(M, D_MODEL).T)
    kT = np.ascontiguousarray(k.reshape(M, D_MODEL).T)
    vT = np.ascontiguousarray(v.reshape(M, D_MODEL).T)

    in_maps = []
    for c in range(N_CORES):
        sl = slice(c * NH, (c + 1) * NH)
        in_maps.append(
            {
                "qT": qT,
                "kT": kT,
                "vT": vT,
                "wqT": np.ascontiguousarray(Wq[sl, :].T * scale),
                "wkT": np.ascontiguousarray(Wk[sl, :].T),
                "wvT": np.ascontiguousarray(Wv[sl, :].T),
                "bq": np.ascontiguousarray(bq[sl, None] * scale),
                "bk": np.ascontiguousarray(bk[sl, None]),
                "bv": np.ascontiguousarray(bv[sl, None]),
                "woT": np.ascontiguousarray(Wo[:, sl].T),
            }
        )

    res = run_bass_kernel_spmd(nc, in_maps, core_ids=list(range(N_CORES)))
    _cache["last_result"] = res

    y = np.zeros((M, D_MODEL), np.float64)
    for c in range(N_CORES):
        y += res.results[c]["y"].astype(np.float64)
    y += bo.astype(np.float64)
    return y.astype(np.float32).reshape(B, S, D_MODEL)
